# revision 30
# baseline (speedup 1.0000x reference)
"""Trainium2 Bass kernel for nn_Matching_Score_word (v3).

Problem: S[i,j] = batch_score(e[i], v[j]) over a 64x64 pair grid;
outputs (S.sum(axis=1), S.sum(axis=0)).

Sharding: data-parallel over j (columns of the pair grid). Core c owns
j in [8c, 8c+8). Each core gets full e + its v slice, computes its 8
columns of S fully on-device; host combines (score_d = sum of per-core
row-partials, score_q = concat of per-core column sums).

v3 design notes (vs v2 baseline at 470us):
- 120-row i*t groups (12x5 words + 1x4, e zero-padded to 1560 it-cols)
  instead of 96-row: ~17% less PE work, all ops uniform [120]-partition.
- softmax-over-n without max subtraction (constant bias -50; randn
  logits have |s| < ~90 so exp(s-50) spans ~[e-140, e+40], safe in f32)
  -> no DVE reduce_max, shorter s->E1 chain.
- recipZ broadcast over t done with stride-0 APs directly in the alpha
  multiplies (no materialized rzx; the v2 gpsimd expand cost 15.7us/j
  and stalled the PE ~13us every j, keeping HAM at K=4/8).
- E2 = exp(G1*E1/sum1) computed as exp(E1R) with E1R = E1*rec4
  prescaled on DVE, so the exp batches across groups (3 ACT ops/j).
- 1/Z and 1/sums via reciprocal_approx_fast (custom DVE op, ~5x faster
  than the iterative reciprocal).
- prod/csq via a bf16 PSUM->SBUF copy; prod on DVE 2x, csq on GpSimd.
- per-j mini-tail: 1/|c| = exp(-0.5*ln(cn2)) on ACT, 1/|e| folded into
  the host-side eTs operand, sumexp accumulated via ACT accum_out.
  The v2 serial 28us end-tail (incl. a 12.9us DVE reciprocal) is gone.
"""

import numpy as np
import os

_SKIP = set(os.environ.get("KSKIP", "").split(",")) - {""}

B, D, T, N = 64, 256, 24, 289
G1, G2 = 4.0, 5.0
NCORES = 8
JPC = B // NCORES          # 8 j per core
NG = 13                    # groups: 12x(5 words) + 1x(4 words)
GI = [5] * 12 + [4]        # words per group
GR = 120                   # rows per group (padded for the last)
IT = NG * GR               # 1560 padded it extent (13*120)
ITR = B * T                # 1536 real it extent
NCH = [128, 128, 33]       # n chunks (289 = 128+128+33)
NOFF = [0, 128, 256]
NB = 289
NB2 = 290                  # even stride for E1/E1R/E2 (DVE 2x needs even FD)
EBIAS = -50.0


def _build_bass():
    import concourse.bass as bass
    import concourse.bacc as bacc
    import concourse.mybir as mybir
    import concourse.tile as tile

    f32 = mybir.dt.float32
    bf16 = mybir.dt.bfloat16
    X = mybir.AxisListType.X
    AF = mybir.ActivationFunctionType

    nc = bacc.Bacc("TRN2", target_bir_lowering=False, debug=False)

    # ---- DRAM I/O (host pre-shapes everything to [partition, cols]) ----
    eH_d = nc.declare_dram_parameter("eH", [128, 2 * IT], bf16, isOutput=False)
    vH_d = nc.declare_dram_parameter("vH", [128, 2 * JPC * NB2], bf16,
                                     isOutput=False)
    vT_d = nc.declare_dram_parameter("vT", [128, JPC * 3 * 256], bf16,
                                     isOutput=False)
    eTs_d = nc.declare_dram_parameter("eTs", [GR, NG * 256], bf16,
                                      isOutput=False)
    o16_d = nc.declare_dram_parameter("ones16", [GR, NG * 64], bf16,
                                      isOutput=False)
    id_d = nc.declare_dram_parameter("identb", [128, 128], bf16, isOutput=False)
    i32_d = nc.declare_dram_parameter("ident32", [64, 64], f32, isOutput=False)
    od_d = nc.declare_dram_parameter("od", [64, 1], f32, isOutput=True)
    oq_d = nc.declare_dram_parameter("oq", [JPC, 1], f32, isOutput=True)

    with tile.TileContext(nc) as tc:
        with (
            nc.allow_low_precision(
                reason="bf16 staging for PE matmuls; end-to-end error "
                       "validated vs fp32 reference"),
            tc.tile_pool(name="const", bufs=1) as cpool,
            tc.tile_pool(name="e1p", bufs=2) as e1p,
            tc.tile_pool(name="e1rp", bufs=2) as e1rp,
            tc.tile_pool(name="e2p", bufs=2) as e2p,
            tc.tile_pool(name="small", bufs=3) as sp,
            tc.tile_pool(name="pcc", bufs=4) as pccp,
            tc.tile_pool(name="pcb", bufs=15) as pcbp,
            tc.tile_pool(name="fin", bufs=1) as fin,
            tc.tile_pool(name="ftmp", bufs=2) as ftmp,
            tc.tile_pool(name="ps_s", bufs=2, space=bass.MemorySpace.PSUM) as ps_s,
            tc.tile_pool(name="ps_z", bufs=2, space=bass.MemorySpace.PSUM) as ps_z,
            tc.tile_pool(name="ps_t", bufs=2, space=bass.MemorySpace.PSUM) as ps_t,
            tc.tile_pool(name="ps_c", bufs=2, space=bass.MemorySpace.PSUM) as ps_c,
        ):
            # ---- load constants into SBUF (ordered by first use) ----
            vHs = [cpool.tile([128, 2 * NB2], bf16, tag=f"vH{j}",
                              name=f"vH{j}") for j in range(JPC)]
            vTs = [cpool.tile([128, 3 * 256], bf16, tag=f"vT{j}",
                              name=f"vT{j}") for j in range(JPC)]

            def load_vj(j):
                for h in range(2):
                    nc.sync.dma_start(
                        vHs[j][:, h * NB2:(h + 1) * NB2],
                        vH_d[:, h * JPC * NB2 + j * NB2:
                             h * JPC * NB2 + (j + 1) * NB2])
                nc.sync.dma_start(vTs[j][:], vT_d[:, j * 768:(j + 1) * 768])

            load_vj(0)
            eH = cpool.tile([128, 2 * IT], bf16)         # cols (h, it)
            nc.sync.dma_start(eH[:, 0:IT], eH_d[:, 0:IT])
            nc.sync.dma_start(eH[:, IT:2 * IT], eH_d[:, IT:2 * IT])
            o16 = cpool.tile([GR, NG * 64], bf16)        # cols (g, i)
            nc.sync.dma_start(o16[:], o16_d[:])
            identb = cpool.tile([128, 128], bf16)
            nc.sync.dma_start(identb[:], id_d[:])
            ident32 = cpool.tile([64, 64], f32)
            nc.sync.dma_start(ident32[:], i32_d[:])
            eTs = cpool.tile([GR, NG * 256], bf16)       # rows (ii,t), cols (g,d)
            nc.sync.dma_start(eTs[:], eTs_d[:])
            for j in range(1, JPC):
                load_vj(j)

            SEc = fin.tile([64, JPC], f32)   # sumexp per (i, j-col)
            DCall = fin.tile([64, JPC * 512], f32)  # staged [dot|cn2] per j
            bias50 = fin.tile([128, 1], f32)
            nc.vector.memset(bias50[:], EBIAS)

            # ---- PE warm-up: ~9us of dummy matmuls during the input DMA
            #      wait so HAM reaches K=8/8 before the real stream starts ----
            wup = fin.tile([128, 512], bf16)
            nc.vector.memset(wup[:], 0.0)
            wps = ps_z.tile([128, 512], f32, tag="dc", bufs=1)
            for k in range(20):
                nc.tensor.matmul(wps[:], wup[:, 0:128], wup[:],
                                 start=(k == 0), stop=(k == 19))

            # AT buffers (manual double buffer so the pad cols can be
            # zeroed once and stay zero: c-matmuls read the full 120-wide
            # window of the last group, whose 24 pad cols must be 0.0)
            ATs = []
            for b_ in range(2):
                at = cpool.tile([128, 3 * IT], bf16, tag=f"AT{b_}")
                atv = at[:].rearrange("p (c it) -> p c it", c=3)
                nc.vector.memset(atv[:, :, ITR:IT], 0.0)
                ATs.append(at)

            for j in range(JPC):
                vH = vHs[j]
                vT = vTs[j]
                AT = ATs[j % 2]
                # ---- pass A: s matmuls + exp(s-50) with accum sums ----
                sums = sp.tile([GR, NG], f32, tag="sums")
                rec = sp.tile([GR, NG], f32, tag="rec")
                E1 = e1p.tile([GR, NG * NB2], bf16, tag="E1")
                E1R = e1rp.tile([GR, NG * NB2], bf16, tag="E1R")
                for g in range(NG):
                    s_ps = ps_s.tile([GR, NB2], f32, tag="s")
                    for h in range(2):
                        lh = slice(h * IT + g * GR, h * IT + (g + 1) * GR)
                        rh = slice(h * NB2, (h + 1) * NB2)
                        if "smm" in _SKIP: break
                        nc.tensor.matmul(s_ps[:], eH[:, lh], vH[:, rh],
                                         start=(h == 0), stop=(h == 1))
                    if "e1" not in _SKIP:
                        nc.scalar.activation(
                            E1[:, g * NB2:(g + 1) * NB2], s_ps[:], AF.Exp,
                            bias=bias50[0:GR, 0:1],
                            accum_out=sums[:, g:g + 1])
                    # batched recips: [0:5] once E1(4) landed, rest at end
                    if g == 4:
                        nc.vector.reciprocal_approx_fast(rec[:, 0:5],
                                                         sums[:, 0:5])
                    elif g == NG - 1:
                        nc.vector.reciprocal_approx_fast(rec[:, 5:NG],
                                                         sums[:, 5:NG])

                # ---- pass B: E1R = G1*E1*rec (DVE 2x), E2 = exp batched ----
                for g in range(NG):
                    if "e1r" in _SKIP: break
                    nc.vector.tensor_scalar(
                        E1R[:, g * NB2:(g + 1) * NB2],
                        E1[:, g * NB2:(g + 1) * NB2], rec[:, g:g + 1], G1,
                        mybir.AluOpType.mult, mybir.AluOpType.mult)
                E2 = e2p.tile([GR, NG * NB2], bf16, tag="E2")
                for b0, b1 in ((0, 5), (5, 10), (10, NG)):
                    if "e2" in _SKIP: break
                    nc.scalar.activation(E2[:, b0 * NB2:b1 * NB2],
                                         E1R[:, b0 * NB2:b1 * NB2], AF.Exp)

                # ---- pass B+: Z (ones matmul) and recipZ transposed ----
                Zps = ps_z.tile([64, NB], f32, tag="z", bufs=1)
                for g in range(NG):
                    if "zmm" in _SKIP: break
                    nc.tensor.matmul(Zps[:],
                                     o16[:, g * 64:(g + 1) * 64],
                                     E2[:, g * NB2:g * NB2 + NB],
                                     start=(g == 0), stop=(g == NG - 1))
                rzf = sp.tile([64, NB], f32, tag="rzf")
                nc.vector.reciprocal_approx_fast(rzf[:], Zps[:])
                rzT = sp.tile([128, 192], bf16, tag="rzTs")  # cols (cc, i)

                # ---- pass C: transpose E2 per group-pair, alpha = E2T*rzT ----
                # (rz transposes are emitted after PE pair 0 so the PE
                #  doesn't stall waiting on the rzf reciprocal)
                for p in range(7):
                    g0 = 2 * p
                    npair = 2 if g0 + 1 < NG else 1
                    niw = GI[g0] + (GI[g0 + 1] if npair == 2 else 0)
                    E2T = ps_t.tile([128, 720], bf16, tag="E2T")
                    for gh in range(npair):
                        g = g0 + gh
                        for cc in range(3):
                            w = NCH[cc]
                            if "tpose" in _SKIP: break
                            nc.tensor.transpose(
                                E2T[0:w, cc * 240 + gh * GR:
                                    cc * 240 + gh * GR + GR],
                                E2[:, g * NB2 + NOFF[cc]:
                                   g * NB2 + NOFF[cc] + w],
                                identb[0:GR, 0:GR])
                    if p == 0:
                        # rz transposes ride after pair 0 (rzf ready by then)
                        rzT_ps = ps_t.tile([128, 192], f32, tag="E2T",
                                           padded_shape=[128, 360])
                        for cc in range(3):
                            w = NCH[cc]
                            nc.tensor.transpose(
                                rzT_ps[0:w, cc * 64:(cc + 1) * 64],
                                rzf[:, NOFF[cc]:NOFF[cc] + w],
                                ident32[0:64, 0:64])
                        nc.vector.tensor_copy(rzT[:, 0:128],
                                              rzT_ps[:, 0:128])
                        nc.vector.tensor_copy(rzT[0:33, 128:192],
                                              rzT_ps[0:33, 128:192])
                    # cols of AT: cc*IT + it ; window it in [120*g0, ...)
                    nreal = niw if g0 < 12 else 4  # last group: 4 real words
                    wcols = nreal * T
                    out01 = AT[:].rearrange("p (c it) -> p c it", c=3)[
                        :, 0:2, g0 * GR:g0 * GR + wcols].rearrange(
                        "p c (i t) -> p c i t", t=T)
                    in01 = E2T[:].rearrange("p (c r) -> p c r", c=3)[
                        :, 0:2, 0:wcols].rearrange(
                        "p c (i t) -> p c i t", t=T)
                    sc01 = rzT[:, 0:128].rearrange(
                        "p (c i) -> p c i", c=2)[
                        :, :, 5 * g0:5 * g0 + nreal].unsqueeze(
                        3).broadcast_to([128, 2, nreal, T])
                    if "alpha" not in _SKIP:
                        nc.vector.tensor_mul(out01, in01, sc01)
                    out2 = AT[0:33].rearrange("p (c it) -> p c it", c=3)[
                        :, 2, g0 * GR:g0 * GR + wcols].rearrange(
                        "p (i t) -> p i t", t=T)
                    in2 = E2T[0:33].rearrange("p (c r) -> p c r", c=3)[
                        :, 2, 0:wcols].rearrange("p (i t) -> p i t", t=T)
                    sc2 = rzT[0:33, 128:192][
                        :, 5 * g0:5 * g0 + nreal].unsqueeze(
                        2).broadcast_to([33, nreal, T])
                    if "alpha" not in _SKIP:
                        nc.vector.tensor_mul(out2, in2, sc2)

                # ---- pass D/E: c matmuls, PCc copy, prod (DVE), csq (gp) ----
                # DC matmuls are emitted after all c matmuls: the PE queue
                # is strict FIFO, so an interleaved dcmm would stall the PE
                # on the DVE/gpsimd elementwise chain every group.
                PCs = []
                for g in range(NG):
                    c2 = ps_c.tile([GR, 256], f32, tag="c2")
                    for cc in range(3):
                        w = NCH[cc]
                        if "cmm" in _SKIP: break
                        nc.tensor.matmul(
                            c2[:],
                            AT[0:w, cc * IT + g * GR:cc * IT + (g + 1) * GR],
                            vT[0:w, cc * 256:(cc + 1) * 256],
                            start=(cc == 0), stop=(cc == 2))
                    PCc = pccp.tile([GR, 256], bf16, tag="PCc")
                    if g % 2 == 0:
                        nc.scalar.copy(PCc[:], c2[:])
                    else:
                        nc.vector.tensor_copy(PCc[:], c2[:])
                    PC = pcbp.tile([GR, 512], bf16, tag="PC")
                    if "prod" not in _SKIP:
                        nc.vector.tensor_mul(
                            PC[:, 0:256], PCc[:],
                            eTs[:, g * 256:(g + 1) * 256])
                    if "csq" not in _SKIP:
                        nc.gpsimd.tensor_mul(PC[:, 256:512], PCc[:], PCc[:])
                    PCs.append(PC)
                DC = ps_z.tile([64, 512], f32, tag="dc", bufs=1)  # [i, (dot|cn2)]
                for g in range(NG):
                    if "dcmm" in _SKIP: break
                    nc.tensor.matmul(
                        DC[:], o16[:, g * 64:(g + 1) * 64], PCs[g][:],
                        start=(g == 0), stop=(g == NG - 1))

                # ---- stage DC to SBUF (tail is batched so the per-j loop
                #      stays pure-Exp: no ACT table switches) ----
                nc.scalar.copy(DCall[:, j * 512:(j + 1) * 512], DC[:])
                if j == 5 or j == JPC - 1:
                    # tail part A (j 0..5) overlaps j6/j7 compute; part B
                    # (j 6..7) is the only serial piece at the end.
                    ja, jb = (0, 6) if j == 5 else (6, JPC)
                    nj = jb - ja
                    dv = DCall[:, ja * 512:jb * 512].rearrange(
                        "p (j q d) -> p j q d", q=2, d=256)
                    lcn = ftmp.tile([64, 6 * 256], f32, tag="lcn")
                    nc.scalar.activation(
                        lcn[:, 0:nj * 256].rearrange(
                            "p (j d) -> p j d", d=256),
                        dv[:, :, 1, :], AF.Ln)
                    rcn = ftmp.tile([64, 6 * 256], f32, tag="rcn")
                    nc.scalar.activation(rcn[:, 0:nj * 256],
                                         lcn[:, 0:nj * 256], AF.Exp,
                                         scale=-0.5)
                    RA = ftmp.tile([64, 6 * 256], f32, tag="RA")
                    nc.vector.tensor_mul(
                        RA[:, 0:nj * 256].rearrange(
                            "p (j d) -> p j d", d=256),
                        dv[:, :, 0, :],
                        rcn[:, 0:nj * 256].rearrange(
                            "p (j d) -> p j d", d=256))
                    xRA = ftmp.tile([64, 6 * 256], f32, tag="xRA")
                    nc.scalar.activation(xRA[:, 0:nj * 256],
                                         RA[:, 0:nj * 256], AF.Exp, scale=G2)
                    nc.vector.reduce_sum(
                        SEc[:, ja:jb],
                        xRA[:, 0:nj * 256].rearrange(
                            "p (j d) -> p j d", d=256), axis=X)

            # ---- end: S = exp(ln(ln(sumexp))/G2), row/col sums ----
            lse = fin.tile([64, JPC], f32)
            nc.scalar.activation(lse[:], SEc[:], AF.Ln)
            lls = fin.tile([64, JPC], f32)
            nc.scalar.activation(lls[:], lse[:], AF.Ln)
            S2 = fin.tile([64, JPC], f32)
            nc.scalar.activation(S2[:], lls[:], AF.Exp, scale=1.0 / G2)
            od_sb = fin.tile([64, 1], f32)
            nc.vector.reduce_sum(od_sb[:], S2[:], axis=X)
            nc.sync.dma_start(od_d[:], od_sb[:])
            # oq needs a sum over i (partitions): PE-transpose S then reduce
            St_ps = ps_z.tile([JPC, 64], f32, tag="dc", bufs=1)
            nc.tensor.transpose(St_ps[:], S2[:], ident32[0:64, 0:64])
            Sq = fin.tile([JPC, 64], f32)
            nc.vector.tensor_copy(Sq[:], St_ps[:])
            oq_sb = fin.tile([JPC, 1], f32)
            nc.vector.reduce_sum(oq_sb[:], Sq[:], axis=X)
            nc.sync.dma_start(oq_d[:], oq_sb[:])

    nc.compile()
    return nc


def _host_inputs(e, v, core):
    """Per-core input map (numpy, shaped/typed as the DRAM params)."""
    import ml_dtypes
    bf = ml_dtypes.bfloat16
    j0 = core * JPC
    vs = v[j0:j0 + JPC]                                     # [8, 256, 289]
    # eH: [128, (h, it)] with it padded 1536->1560 (zeros)
    eW = e.transpose(1, 0, 2).reshape(D, ITR)               # [256, 1536]
    eWp = np.zeros((D, IT), dtype=np.float32)
    eWp[:, :ITR] = eW
    eH = np.concatenate([eWp[:128], eWp[128:]], axis=1).astype(bf)  # [128, 2*IT]
    # vH: [128, (h, j, n)] with n padded 289->290 (zero col per (h, j))
    vWh = vs.transpose(1, 0, 2)                             # [256, 8, 289]
    vWp = np.zeros((D, JPC, NB2), dtype=np.float32)
    vWp[:, :, :N] = vWh
    vWp = vWp.reshape(D, JPC * NB2)
    vH = np.ascontiguousarray(
        np.concatenate([vWp[:128], vWp[128:]], axis=1),
        dtype=np.float32).astype(bf)
    # vT: [128, (j, cc, d)] n-padded to 3*128
    vTt = vs.transpose(0, 2, 1)                             # [8, 289, 256]
    vTp = np.zeros((JPC, 3 * 128, 256), dtype=np.float32)
    vTp[:, :N, :] = vTt
    vT = np.ascontiguousarray(
        vTp.reshape(JPC * 3, 128, 256).transpose(1, 0, 2).reshape(128, -1)
    ).astype(bf)
    # eTs: rows (ii, t) within group, cols (g, d); scaled by 1/|e|;
    # last group rows 96:120 zero.
    renA = 1.0 / np.sqrt((e.astype(np.float32) ** 2).sum(axis=2))  # [64, 256]
    eTs = np.zeros((GR, NG * 256), dtype=np.float32)
    for g in range(NG):
        for ii in range(GI[g]):
            i = 5 * g + ii
            eTs[ii * T:(ii + 1) * T, g * 256:(g + 1) * 256] = (
                e[i].T * renA[i][None, :])
    # o16: block g: [120, 64] with 1 at (r, 5g + r//24); pad rows zero
    o16 = np.zeros((NG, GR, 64), dtype=np.float32)
    for g in range(NG):
        for ii in range(GI[g]):
            o16[g, ii * T:(ii + 1) * T, 5 * g + ii] = 1.0
    o16 = np.ascontiguousarray(
        o16.transpose(1, 0, 2).reshape(GR, NG * 64)).astype(bf)
    identb = np.eye(128, dtype=np.float32).astype(bf)
    return {
        "eH": eH, "vH": vH, "vT": vT, "eTs": eTs.astype(bf),
        "ones16": o16, "identb": identb,
        "ident32": np.eye(64, dtype=np.float32),
    }


_CACHE = {}


def kernel(e, v, _trace=False):
    from concourse.bass_utils import run_bass_kernel_spmd

    e = np.asarray(e, dtype=np.float32)
    v = np.asarray(v, dtype=np.float32)
    if "nc" not in _CACHE:
        _CACHE["nc"] = _build_bass()
    nc = _CACHE["nc"]
    in_maps = [_host_inputs(e, v, c) for c in range(NCORES)]
    res = run_bass_kernel_spmd(nc, in_maps, list(range(NCORES)), trace=_trace)
    od = np.zeros(64, dtype=np.float32)
    oq = np.zeros(64, dtype=np.float32)
    for c in range(NCORES):
        od += res.results[c]["od"].reshape(64)
        oq[c * JPC:(c + 1) * JPC] = res.results[c]["oq"].reshape(JPC)
    if _trace:
        return (od, oq), res
    return (od, oq)
